# revision 32
# baseline (speedup 1.0000x reference)
# BitConvBlock Trainium2 kernel: LayerNorm -> activation int8-quant ->
# ternary weight quant -> conv1d(K=3, pad 1) -> rescale.
#
# Sharding: data-parallel over batch (B=8) across the 8 NeuronCores; every
# core gets one batch element plus replicated W / ln params, computes its
# full [T, C] output slice, host stacks the results. The host also hands W
# to each core pre-permuted to [C_in, K, C_out] (a pure layout change) so
# the ternary-quantized operand lands in matmul orientation directly.
#
# Numerics: x_q is an integer in [-127, 127] and w_q is in {-1, 0, 1};
# both are exact in bf16 and every partial sum is < 2^24, so bf16 matmuls
# with fp32 PSUM accumulation reproduce the fp32 reference conv almost
# exactly. Rounding uses the fp32 +-1.5*2^23 trick (round-to-nearest-even,
# matching jnp.round). Deliberate approximations, all well inside the
# 2e-2 budget: xhat is stored as fp16/bf16 (the quantization grid shifts
# by ~2^-9 relative, flipping a small fraction of rounds by +-1 LSB) and
# the rsqrt Newton correction is dropped (ACT Sqrt + DVE reciprocal are
# ~1ulp).
#
# Schedule: single streamed pass over x (two DMA queues) computes LN stats
# (DVE), |xhat| in fp16 (ACT, in groups of four with a batched rsqrt
# chain), the per-channel running max (DVE), and PE-transposes each xhat
# tile straight into the bf16 operand buffer xqt_all (the PE is otherwise
# idle in the prologue and this also keeps its HAM clock warm). W streams
# on a third queue: one scan pass for beta_w = mean|W| (ACT abs+accum),
# then a second streamed pass through a small ring for u = rne(W/beta)
# (DVE) and Sign (ACT) directly into wqt_all. Scales then reduce amx via
# 8 PE transposes into column form -- in the transposed domain the
# activation scale A is a per-partition column, so no broadcast of A is
# needed at all; only the output rescale r is broadcast to [128, C].
# Quantization (u = xhat*A + RC; xq = u - RC) runs in place over xqt_all
# in [128, TQ] chunks, one chunk-group ahead of the consume matmuls, and
# overlaps the main phase where ACT/DVE are otherwise idle.

import numpy as np

import concourse.bacc as bacc
import concourse.bass as bass
import concourse.mybir as mybir
import concourse.tile as tile
from concourse.bass_utils import run_bass_kernel_spmd
from concourse.masks import make_identity

F32 = mybir.dt.float32
F16 = mybir.dt.float16
BF16 = mybir.dt.bfloat16
AX = mybir.AxisListType
OP = mybir.AluOpType
AF = mybir.ActivationFunctionType

QP = 127.0
EPS_LN = 1e-5
EPS_CLAMP = 1e-5
RC = 1.5 * 2.0**23  # fp32 round-to-nearest-even magic constant
N_CORES = 8
KW = 3  # conv kernel width


def build_kernel(T, C, beta_zero, n_cores=N_CORES):
    """Build and compile the per-core Bass program for x:[T,C] W:[C,3,C]."""
    assert T % 128 == 0 and C % 128 == 0
    NT = T // 128            # time tiles
    NCC = C // 128           # channel chunks of 128
    OSL = min(512, C)        # output-channel slab (one PSUM bank)
    NH = C // OSL            # slabs per tile
    TQ = min(1024, T)        # quantize chunk along T
    NQ = T // TQ
    NTQ = TQ // 128          # time tiles per chunk
    SUB = min(512, C)        # bn_stats subgroup
    NS = C // SUB
    XPAD = 16                # left pad in xqT keeps windows off the edge
    W_COUNT = float(C * C * KW)
    XG = 4                   # X1 group size (tiles per rsqrt batch)
    NG = NT // XG

    # W-pipeline emission schedule (x-group index -> o-tiles).
    # Pass 1 (abs scan) chases the W DMA arrivals; beta after the last;
    # pass 2 (re-read + quantize) spreads over the remaining groups.
    W1DMA_AT = {0: [0, 1], 1: [2, 3], 2: [4, 5], 3: [6, 7]}
    W1_AT = {1: [0], 2: [1], 3: [2, 3], 4: [4, 5], 5: [6, 7]}
    BETA_G = 5
    W2_AT = {5: [0, 1], 6: [2, 3], 7: [4, 5]}
    W2_TAIL = [6, 7]

    nc = bacc.Bacc("TRN2", target_bir_lowering=False, debug=False,
                   num_devices=n_cores)
    x_d = nc.dram_tensor("x", [T, C], F32, kind="ExternalInput")
    g_d = nc.dram_tensor("ln_gamma", [C], F32, kind="ExternalInput")
    b_d = nc.dram_tensor("ln_beta", [C], F32, kind="ExternalInput")
    w_d = nc.dram_tensor("W", [C, KW, C], F32, kind="ExternalInput")
    out_d = nc.dram_tensor("out", [T, C], F32, kind="ExternalOutput")

    with tile.TileContext(nc) as tc:
        import contextlib
        with contextlib.ExitStack() as ctx:
            const = ctx.enter_context(tc.tile_pool(name="const", bufs=1))
            identb = const.tile([128, 128], F32)
            make_identity(nc, identb[:])
            identh = const.tile([128, 128], BF16)
            nc.vector.tensor_copy(identh[:], identb[:])
            ones_row = const.tile([1, 128], F32)
            nc.vector.memset(ones_row[:], 1.0)
            ones_col = const.tile([128, 1], F32)
            nc.vector.memset(ones_col[:], 1.0)
            rcp_col = const.tile([128, 1], F32)
            nc.vector.memset(rcp_col[:], RC)
            rcn_col = const.tile([128, 1], F32)
            nc.vector.memset(rcn_col[:], -RC)
            eps_col = const.tile([128, 1], F32)
            nc.vector.memset(eps_col[:], EPS_LN)

            mv_all = const.tile([128, NT, 2], F32)    # per-tile mean/var
            rsig_all = const.tile([128, NT], F32)
            nmr_all = const.tile([128, NT], F32)      # -mu * rsig
            wabs = const.tile([128, NCC], F32)
            binv_col = const.tile([128, 1], F32)
            beta_col = const.tile([128, 1], F32)

            # gamma/beta reshaped [128, NCC]: element (p, j) = param[j*128+p]
            g_mat = const.tile([128, NCC], F32)
            nc.gpsimd.dma_start(out=g_mat[:],
                                in_=g_d.ap().rearrange("(j p) -> p j", p=128))
            if not beta_zero:
                b_mat = const.tile([128, NCC], F32, name="b_mat")
                nc.gpsimd.dma_start(out=b_mat[:],
                                    in_=b_d.ap().rearrange("(j p) -> p j", p=128))

            # bf16 extrema accumulators (2x DVE rate; ~2^-9 scale error)
            amx_t = const.tile([128, C], BF16)
            nc.vector.memset(amx_t[:], -3.0e38)
            amn_t = const.tile([128, C], BF16, name="amn_t")
            nc.vector.memset(amn_t[:], 3.0e38)

            r_b = const.tile([128, C], F32)           # output rescale bcast

            # persistent operands: transposed weights and activations
            wqt_all = const.tile([128, NCC, KW, C], BF16)
            xqt_all = const.tile([128, NCC, 2 * XPAD + T], BF16)

            xin = ctx.enter_context(tc.tile_pool(name="xin", bufs=6))
            xhat_p = ctx.enter_context(tc.tile_pool(name="xhat", bufs=8))
            win_p = ctx.enter_context(tc.tile_pool(name="win", bufs=2))
            dump_p = ctx.enter_context(tc.tile_pool(name="dump", bufs=1))
            u_p = ctx.enter_context(tc.tile_pool(name="u", bufs=2))
            yout = ctx.enter_context(tc.tile_pool(name="yout", bufs=2))
            small = ctx.enter_context(tc.tile_pool(name="small", bufs=2))
            st_p = ctx.enter_context(tc.tile_pool(name="st", bufs=2))
            grp_p = ctx.enter_context(tc.tile_pool(name="grp", bufs=3))

            psum_mm = ctx.enter_context(
                tc.tile_pool(name="psum_mm", bufs=6, space="PSUM"))
            psum_ms = ctx.enter_context(
                tc.tile_pool(name="psum_ms", bufs=2, space="PSUM"))

            def ptile():
                return psum_ms.tile([128, 512], F32, tag="ms", name="pms")

            def ptile16():
                return psum_ms.tile([128, 512], BF16, tag="ms", name="pms16")

            # ---- W pass 1: stream all tiles through a small ring ---------
            w1tiles = {}

            def w1_dma(ot):
                wt = win_p.tile([128, KW, C], F32, tag="wt", name="wt")
                w1tiles[ot] = wt
                nc.gpsimd.dma_start(out=wt[:],
                                    in_=w_d[ot * 128:(ot + 1) * 128, :, :])

            def w_abs(ot):
                # |W| row-sums on ACT; main output discarded into a bf16
                # scratch slot (the accumulator itself is fp32)
                wt = w1tiles.pop(ot)
                dump = dump_p.tile([128, KW, C], BF16, tag="dump", name="dump")
                nc.scalar.activation(dump[:], wt[:], AF.Abs,
                                     accum_out=wabs[:, ot:ot + 1])

            def beta_block():
                # beta_w = max(mean|W|, eps); binv = 1/beta
                wsum = small.tile([128, 1], F32, tag="wsum", name="wsum")
                nc.vector.reduce_sum(wsum[:], wabs[:], axis=AX.X)
                ps1 = psum_ms.tile([1, 1], F32, tag="ms", name="ps1")
                nc.tensor.matmul(ps1[:], ones_col[:], wsum[:], start=True,
                                 stop=True)
                bsc = small.tile([1, 1], F32, tag="bsc", name="bsc")
                nc.vector.tensor_scalar(bsc[:], ps1[:], 1.0 / W_COUNT,
                                        EPS_CLAMP, op0=OP.mult, op1=OP.max)
                psb = psum_ms.tile([128, 1], F32, tag="ms", name="psb")
                nc.tensor.matmul(psb[:], ones_row[:], bsc[:], start=True,
                                 stop=True)
                nc.vector.tensor_copy(beta_col[:], psb[:])
                nc.vector.reciprocal(binv_col[:], beta_col[:])

            def w_quant(ot):
                # second streamed read; u = rne(w/beta) + RC on DVE in the
                # ring slot; Sign on ACT writes wqt_all directly (W arrives
                # host-transposed, so no PE transposes are needed)
                wt = win_p.tile([128, KW, C], F32, tag="wt", name="wt")
                nc.gpsimd.dma_start(out=wt[:],
                                    in_=w_d[ot * 128:(ot + 1) * 128, :, :])
                nc.vector.tensor_scalar(wt[:], wt[:], binv_col[:], RC,
                                        op0=OP.mult, op1=OP.add)
                nc.scalar.activation(wqt_all[:, ot], wt[:], AF.Sign,
                                     bias=rcn_col[:], scale=1.0)

            # ============ Pass X1: stats + |xhat| + transpose-in ==========
            # Software-pipelined groups of XG tiles: DMA+stats of group g
            # are issued before the rsqrt chain / xhat of group g-1 so the
            # in-order engine queues never convoy on the latency chain.
            xts = {}
            xhs = {}

            def x_dma_stats(g):
                for u in range(XG):
                    it = g * XG + u
                    xt = xin.tile([128, C], F32, tag="xt", name="xt")
                    xts[it] = xt
                    nc.sync.dma_start(out=xt[:],
                                      in_=x_d[it * 128:(it + 1) * 128, :])
                    st6 = st_p.tile([128, NS, 6], F32)
                    for sb in range(NS):
                        nc.vector.bn_stats(st6[:, sb, :],
                                           xt[:, sb * SUB:(sb + 1) * SUB])
                    nc.vector.bn_aggr(mv_all[:, it, :], st6[:])

            def x_chain(g):
                # rsig = 1/sqrt(var+eps): +eps folds into the Sqrt bias so
                # the chain is one ACT op + three small DVE ops; runs at
                # high priority so W-pipeline work never delays the
                # latency-critical path that recycles the x input slots
                gs = slice(g * XG, (g + 1) * XG)
                with tc.high_priority():
                    s0 = grp_p.tile([128, XG], F32, tag="g2", name="g2")
                    nc.scalar.activation(s0[:], mv_all[:, gs, 1], AF.Sqrt,
                                         bias=eps_col[:], scale=1.0)
                    nc.vector.reciprocal(rsig_all[:, gs], s0[:])
                    mr = grp_p.tile([128, XG], F32, tag="g2", name="g2")
                    nc.vector.tensor_tensor(mr[:], mv_all[:, gs, 0],
                                            rsig_all[:, gs], op=OP.mult)
                    nc.vector.tensor_scalar_mul(nmr_all[:, gs], mr[:], -1.0)

            def x_finish(g):
                for u in range(XG):
                    it = g * XG + u
                    xt = xts.pop(it)
                    xh = xhat_p.tile([128, C], BF16, tag="xh", name="xh")
                    nc.scalar.activation(
                        xh[:], xt[:], AF.Identity,
                        bias=nmr_all[:, it:it + 1],
                        scale=rsig_all[:, it:it + 1])
                    nc.vector.tensor_tensor(amx_t[:], amx_t[:], xh[:],
                                            op=OP.max)
                    nc.vector.tensor_tensor(amn_t[:], amn_t[:], xh[:],
                                            op=OP.min)
                    xhs[it] = xh

            def x_transpose(g):
                # one SBUF->SBUF xbar transpose per tile lands xhat in the
                # operand buffer (fabric bandwidth, no ACT/DVE/PE cost);
                # emitted two groups late so the wait is already satisfied
                # and the sync queue never head-blocks the x reads
                for u in range(XG):
                    it = g * XG + u
                    xh = xhs.pop(it)
                    nc.sync.dma_start_transpose(
                        xqt_all[:, :, XPAD + it * 128:XPAD + (it + 1) * 128],
                        xh[:])

            for ot in W1DMA_AT.get(0, []):
                w1_dma(ot)
            x_dma_stats(0)
            x_chain(0)
            for g in range(1, NG):
                x_dma_stats(g)
                x_finish(g - 1)
                if g >= 2:
                    x_transpose(g - 2)
                x_chain(g)
                for ot in W1DMA_AT.get(g, []):
                    w1_dma(ot)
                for ot in W1_AT.get(g, []):
                    w_abs(ot)
                if g == BETA_G:
                    beta_block()
                for ot in W2_AT.get(g, []):
                    w_quant(ot)
            x_finish(NG - 1)
            x_transpose(NG - 2)
            x_transpose(NG - 1)
            for ot in W2_TAIL:
                w_quant(ot)

            # ============ per-channel scales (column form) ================
            Mx = small.tile([128, NCC], F32, tag="Mx", name="Mx")
            Mn = small.tile([128, NCC], F32, tag="Mn", name="Mn")
            for j in range(NCC):
                pmx = ptile16()
                nc.tensor.transpose(pmx[:, 0:128],
                                    amx_t[:, j * 128:(j + 1) * 128], identh[:])
                nc.vector.tensor_reduce(Mx[:, j:j + 1], pmx[:, 0:128],
                                        axis=AX.X, op=OP.max)
                pmn = ptile16()
                nc.tensor.transpose(pmn[:, 0:128],
                                    amn_t[:, j * 128:(j + 1) * 128],
                                    identh[:])
                nc.vector.tensor_reduce(Mn[:, j:j + 1], pmn[:, 0:128],
                                        axis=AX.X, op=OP.min)
            # batched endpoint math on [128, NCC]
            t1 = small.tile([128, NCC], F32, tag="t1", name="t1")
            t2 = small.tile([128, NCC], F32, tag="t2", name="t2")
            nc.vector.tensor_tensor(t1[:], g_mat[:], Mx[:], op=OP.mult)
            nc.vector.tensor_tensor(t2[:], g_mat[:], Mn[:], op=OP.mult)
            if not beta_zero:
                nc.vector.tensor_tensor(t1[:], t1[:], b_mat[:], op=OP.add)
                nc.vector.tensor_tensor(t2[:], t2[:], b_mat[:], op=OP.add)
            m1 = small.tile([128, NCC], F32, tag="m1", name="m1")
            nc.vector.tensor_tensor(m1[:], t1[:], t2[:], op=OP.max)
            nc.vector.tensor_scalar_mul(t2[:], t2[:], -1.0)
            nc.vector.tensor_scalar_mul(t1[:], t1[:], -1.0)
            nc.vector.tensor_tensor(m1[:], m1[:], t2[:], op=OP.max)
            nc.vector.tensor_tensor(m1[:], m1[:], t1[:], op=OP.max)  # amax
            nc.vector.tensor_scalar_max(m1[:], m1[:], EPS_CLAMP)     # gamma_q
            ginv = small.tile([128, NCC], F32, tag="ginv", name="ginv")
            nc.vector.reciprocal(ginv[:], m1[:])
            sc_m = small.tile([128, NCC], F32, tag="scm", name="scm")
            nc.vector.tensor_scalar_mul(sc_m[:], ginv[:], QP)
            scinv = small.tile([128, NCC], F32, tag="sci", name="sci")
            nc.vector.reciprocal(scinv[:], sc_m[:])
            # A (and B) are per-partition columns in the transposed domain
            A_m = small.tile([128, NCC], F32, tag="Am", name="Am")
            nc.vector.tensor_tensor(A_m[:], g_mat[:], sc_m[:], op=OP.mult)
            r_m = small.tile([128, NCC], F32, tag="rm", name="rm")
            nc.vector.tensor_scalar_mul(r_m[:], scinv[:], beta_col[:])
            BR_m = small.tile([128, NCC], F32, tag="BRm", name="BRm")
            if not beta_zero:
                nc.vector.tensor_tensor(BR_m[:], b_mat[:], sc_m[:],
                                        op=OP.mult)
                nc.vector.tensor_scalar_add(BR_m[:], BR_m[:], RC)
            # broadcast r to [128, C] via per-column transpose + K=1 matmul
            for j in range(NCC):
                prow = ptile()
                nc.tensor.transpose(prow[0:1, 0:128], r_m[:, j:j + 1],
                                    identb[:])
                rw = small.tile([1, 128], F32, tag="rw", name="rw")
                nc.vector.tensor_copy(rw[:], prow[0:1, 0:128])
                pbc = ptile()
                nc.tensor.matmul(pbc[:, 0:128], ones_row[:], rw[:],
                                 start=True, stop=True)
                nc.vector.tensor_copy(r_b[:, j * 128:(j + 1) * 128],
                                      pbc[:, 0:128])

            # zero the one-column halo each side of the time axis
            for j in range(NCC):
                nc.vector.memset(xqt_all[:, j, XPAD - 1:XPAD], 0.0)
                nc.vector.memset(xqt_all[:, j, XPAD + T:XPAD + T + 1], 0.0)

            # ============ quantize chunks + conv matmuls ==================
            def quantize(q):
                # in place over xqt_all: u = xhat*A (+B) + RC on ACT;
                # xq = u - RC on DVE (bf16 out)
                for j in range(NCC):
                    sl = xqt_all[:, j, XPAD + q * TQ:XPAD + (q + 1) * TQ]
                    u = u_p.tile([128, TQ], F32, tag="u", name="u")
                    nc.scalar.activation(
                        u[:], sl, AF.Identity,
                        bias=rcp_col[:] if beta_zero else BR_m[:, j:j + 1],
                        scale=A_m[:, j:j + 1])
                    nc.vector.tensor_scalar_add(sl, u[:], -RC)

            def consume(q):
                for itq in range(NTQ):
                    it = q * NTQ + itq
                    pss = [psum_mm.tile([128, OSL], F32, tag="mm", name="pmm")
                           for _ in range(NH)]
                    for j in range(NCC):
                        for k in range(KW):
                            lhsT = xqt_all[:, j, XPAD + it * 128 + k - 1:
                                           XPAD + it * 128 + k - 1 + 128]
                            first = (j == 0 and k == 0)
                            last = (j == NCC - 1 and k == KW - 1)
                            for h in range(NH):
                                nc.tensor.matmul(
                                    pss[h][:], lhsT,
                                    wqt_all[:, j, k, h * OSL:(h + 1) * OSL],
                                    start=first, stop=last)
                    for h in range(NH):
                        yt = yout.tile([128, OSL], F32, tag="yt", name="yt")
                        nc.vector.tensor_tensor(
                            yt[:], pss[h][:], r_b[:, h * OSL:(h + 1) * OSL],
                            op=OP.mult)
                        nc.gpsimd.dma_start(
                            out=out_d[it * 128:(it + 1) * 128,
                                      h * OSL:(h + 1) * OSL],
                            in_=yt[:])

            quantize(0)
            if NQ > 1:
                quantize(1)
            for q in range(NQ):
                consume(q)
                if q + 2 < NQ:
                    quantize(q + 2)

    nc.compile()
    return nc


_NC_CACHE = {}


def _get_nc(T, C, beta_zero):
    key = (T, C, beta_zero)
    if key not in _NC_CACHE:
        _NC_CACHE[key] = build_kernel(T, C, beta_zero)
    return _NC_CACHE[key]


def run(inputs, trace=False):
    """Run the SPMD kernel; returns (output [B,T,C], BassKernelResults)."""
    x = np.ascontiguousarray(np.asarray(inputs["x"], dtype=np.float32))
    g = np.ascontiguousarray(np.asarray(inputs["ln_gamma"], dtype=np.float32))
    b = np.ascontiguousarray(np.asarray(inputs["ln_beta"], dtype=np.float32))
    W = np.asarray(inputs["W"], dtype=np.float32)
    B, T, C = x.shape
    assert B == N_CORES, f"expected batch {N_CORES}, got {B}"
    beta_zero = bool(np.all(b == 0.0))
    nc = _get_nc(T, C, beta_zero)
    # pure layout permute: supply W as [C_in, K, C_out] so the quantized
    # operand lands in matmul orientation with no on-chip transposes
    W_T = np.ascontiguousarray(W.transpose(1, 2, 0))
    in_maps = [
        {"x": np.ascontiguousarray(x[i]), "ln_gamma": g, "ln_beta": b,
         "W": W_T}
        for i in range(B)
    ]
    res = run_bass_kernel_spmd(nc, in_maps, core_ids=list(range(N_CORES)),
                               trace=trace)
    out = np.stack([res.results[i]["out"] for i in range(B)], axis=0)
    return out, res


def kernel(**inputs) -> np.ndarray:
    out, _ = run(inputs)
    return out


# revision 33
# speedup vs baseline: 1.0470x; 1.0470x over previous
# BitConvBlock Trainium2 kernel: LayerNorm -> activation int8-quant ->
# ternary weight quant -> conv1d(K=3, pad 1) -> rescale.
#
# Sharding: data-parallel over batch (B=8) across the 8 NeuronCores; every
# core gets one batch element plus replicated W / ln params, computes its
# full [T, C] output slice, host stacks the results. The host also hands W
# to each core pre-permuted to [C_in, K, C_out] (a pure layout change) so
# the ternary-quantized operand lands in matmul orientation directly.
#
# Numerics: x_q is an integer in [-127, 127] and w_q is in {-1, 0, 1};
# both are exact in bf16 and every partial sum is < 2^24, so bf16 matmuls
# with fp32 PSUM accumulation reproduce the fp32 reference conv almost
# exactly. Rounding uses the fp32 +-1.5*2^23 trick (round-to-nearest-even,
# matching jnp.round). Deliberate approximations, all well inside the
# 2e-2 budget: xhat is stored as fp16/bf16 (the quantization grid shifts
# by ~2^-9 relative, flipping a small fraction of rounds by +-1 LSB) and
# the rsqrt Newton correction is dropped (ACT Sqrt + DVE reciprocal are
# ~1ulp).
#
# Schedule: single streamed pass over x (two DMA queues) computes LN stats
# (DVE), |xhat| in fp16 (ACT, in groups of four with a batched rsqrt
# chain), the per-channel running max (DVE), and PE-transposes each xhat
# tile straight into the bf16 operand buffer xqt_all (the PE is otherwise
# idle in the prologue and this also keeps its HAM clock warm). W streams
# on a third queue: one scan pass for beta_w = mean|W| (ACT abs+accum),
# then a second streamed pass through a small ring for u = rne(W/beta)
# (DVE) and Sign (ACT) directly into wqt_all. Scales then reduce amx via
# 8 PE transposes into column form -- in the transposed domain the
# activation scale A is a per-partition column, so no broadcast of A is
# needed at all; only the output rescale r is broadcast to [128, C].
# Quantization (u = xhat*A + RC; xq = u - RC) runs in place over xqt_all
# in [128, TQ] chunks, one chunk-group ahead of the consume matmuls, and
# overlaps the main phase where ACT/DVE are otherwise idle.

import numpy as np

import concourse.bacc as bacc
import concourse.bass as bass
import concourse.mybir as mybir
import concourse.tile as tile
from concourse.bass_utils import run_bass_kernel_spmd
from concourse.masks import make_identity

F32 = mybir.dt.float32
F16 = mybir.dt.float16
BF16 = mybir.dt.bfloat16
AX = mybir.AxisListType
OP = mybir.AluOpType
AF = mybir.ActivationFunctionType

QP = 127.0
EPS_LN = 1e-5
EPS_CLAMP = 1e-5
RC = 1.5 * 2.0**23  # fp32 round-to-nearest-even magic constant
N_CORES = 8
KW = 3  # conv kernel width


def build_kernel(T, C, beta_zero, n_cores=N_CORES):
    """Build and compile the per-core Bass program for x:[T,C] W:[C,3,C]."""
    assert T % 128 == 0 and C % 128 == 0
    NT = T // 128            # time tiles
    NCC = C // 128           # channel chunks of 128
    OSL = min(512, C)        # output-channel slab (one PSUM bank)
    NH = C // OSL            # slabs per tile
    TQ = min(1024, T)        # quantize chunk along T
    NQ = T // TQ
    NTQ = TQ // 128          # time tiles per chunk
    SUB = min(512, C)        # bn_stats subgroup
    NS = C // SUB
    XPAD = 16                # left pad in xqT keeps windows off the edge
    W_COUNT = float(C * C * KW)
    XG = 4                   # X1 group size (tiles per rsqrt batch)
    NG = NT // XG

    # W-pipeline emission schedule (x-group index -> o-tiles).
    # Pass 1 (abs scan) chases the W DMA arrivals; beta after the last;
    # pass 2 (re-read + quantize) spreads over the remaining groups.
    W1_AT = {1: [0], 2: [1], 3: [2, 3], 4: [4, 5], 5: [6, 7]}
    BETA_G = 5
    W2_AT = {5: [0, 1], 6: [2, 3], 7: [4, 5]}
    W2_TAIL = [6, 7]

    nc = bacc.Bacc("TRN2", target_bir_lowering=False, debug=False,
                   num_devices=n_cores)
    x_d = nc.dram_tensor("x", [T, C], F32, kind="ExternalInput")
    g_d = nc.dram_tensor("ln_gamma", [C], F32, kind="ExternalInput")
    b_d = nc.dram_tensor("ln_beta", [C], F32, kind="ExternalInput")
    w_d = nc.dram_tensor("W", [C, KW, C], F32, kind="ExternalInput")
    out_d = nc.dram_tensor("out", [T, C], F32, kind="ExternalOutput")

    with tile.TileContext(nc) as tc:
        import contextlib
        with contextlib.ExitStack() as ctx:
            const = ctx.enter_context(tc.tile_pool(name="const", bufs=1))
            identb = const.tile([128, 128], F32)
            make_identity(nc, identb[:])
            identh = const.tile([128, 128], F16)
            nc.vector.tensor_copy(identh[:], identb[:])
            ones_row = const.tile([1, 128], F32)
            nc.vector.memset(ones_row[:], 1.0)
            ones_col = const.tile([128, 1], F32)
            nc.vector.memset(ones_col[:], 1.0)
            rcp_col = const.tile([128, 1], F32)
            nc.vector.memset(rcp_col[:], RC)
            rcn_col = const.tile([128, 1], F32)
            nc.vector.memset(rcn_col[:], -RC)
            eps_col = const.tile([128, 1], F32)
            nc.vector.memset(eps_col[:], EPS_LN)

            mv_all = const.tile([128, NT, 2], F32)    # per-tile mean/var
            rsig_all = const.tile([128, NT], F32)
            nmr_all = const.tile([128, NT], F32)      # -mu * rsig
            wabs = const.tile([128, NCC], F32)
            binv_col = const.tile([128, 1], F32)
            beta_col = const.tile([128, 1], F32)

            # gamma/beta reshaped [128, NCC]: element (p, j) = param[j*128+p]
            g_mat = const.tile([128, NCC], F32)
            nc.gpsimd.dma_start(out=g_mat[:],
                                in_=g_d.ap().rearrange("(j p) -> p j", p=128))
            if not beta_zero:
                b_mat = const.tile([128, NCC], F32, name="b_mat")
                nc.gpsimd.dma_start(out=b_mat[:],
                                    in_=b_d.ap().rearrange("(j p) -> p j", p=128))

            # fp16 extrema accumulators (2x DVE rate; ~2^-12 scale error)
            amx_t = const.tile([128, C], F16)
            nc.vector.memset(amx_t[:], -65504.0)
            amn_t = const.tile([128, C], F16, name="amn_t")
            nc.vector.memset(amn_t[:], 65504.0)

            r_b = const.tile([128, C], F32)           # output rescale bcast

            # persistent operands: transposed weights and activations
            wqt_all = const.tile([128, NCC, KW, C], BF16)
            xqt_all = const.tile([128, NCC, 2 * XPAD + T], BF16)

            xin = ctx.enter_context(tc.tile_pool(name="xin", bufs=6))
            xhat_p = ctx.enter_context(tc.tile_pool(name="xhat", bufs=3))
            win_p = ctx.enter_context(tc.tile_pool(name="win", bufs=2))
            dump_p = ctx.enter_context(tc.tile_pool(name="dump", bufs=1))
            u_p = ctx.enter_context(tc.tile_pool(name="u", bufs=2))
            yout = ctx.enter_context(tc.tile_pool(name="yout", bufs=2))
            small = ctx.enter_context(tc.tile_pool(name="small", bufs=2))
            st_p = ctx.enter_context(tc.tile_pool(name="st", bufs=2))
            grp_p = ctx.enter_context(tc.tile_pool(name="grp", bufs=3))

            psum_mm = ctx.enter_context(
                tc.tile_pool(name="psum_mm", bufs=6, space="PSUM"))
            psum_ms = ctx.enter_context(
                tc.tile_pool(name="psum_ms", bufs=2, space="PSUM"))

            def ptile():
                return psum_ms.tile([128, 512], F32, tag="ms", name="pms")

            def ptile16():
                return psum_ms.tile([128, 512], F16, tag="ms", name="pms16")

            # ---- W pass 1: stream all tiles through a small ring ---------
            w1tiles = {}
            for ot in range(NCC):
                wt = win_p.tile([128, KW, C], F32, tag="wt", name="wt")
                w1tiles[ot] = wt
                nc.gpsimd.dma_start(out=wt[:],
                                    in_=w_d[ot * 128:(ot + 1) * 128, :, :])

            def w_abs(ot):
                # |W| row-sums on ACT; main output discarded into a bf16
                # scratch slot (the accumulator itself is fp32)
                wt = w1tiles.pop(ot)
                dump = dump_p.tile([128, KW, C], BF16, tag="dump", name="dump")
                nc.scalar.activation(dump[:], wt[:], AF.Abs,
                                     accum_out=wabs[:, ot:ot + 1])

            def beta_block():
                # beta_w = max(mean|W|, eps); binv = 1/beta
                wsum = small.tile([128, 1], F32, tag="wsum", name="wsum")
                nc.vector.reduce_sum(wsum[:], wabs[:], axis=AX.X)
                ps1 = psum_ms.tile([1, 1], F32, tag="ms", name="ps1")
                nc.tensor.matmul(ps1[:], ones_col[:], wsum[:], start=True,
                                 stop=True)
                bsc = small.tile([1, 1], F32, tag="bsc", name="bsc")
                nc.vector.tensor_scalar(bsc[:], ps1[:], 1.0 / W_COUNT,
                                        EPS_CLAMP, op0=OP.mult, op1=OP.max)
                psb = psum_ms.tile([128, 1], F32, tag="ms", name="psb")
                nc.tensor.matmul(psb[:], ones_row[:], bsc[:], start=True,
                                 stop=True)
                nc.vector.tensor_copy(beta_col[:], psb[:])
                nc.vector.reciprocal(binv_col[:], beta_col[:])

            def w_quant(ot):
                # second streamed read; u = rne(w/beta) + RC on DVE in the
                # ring slot; Sign on ACT writes wqt_all directly (W arrives
                # host-transposed, so no PE transposes are needed)
                wt = win_p.tile([128, KW, C], F32, tag="wt", name="wt")
                nc.gpsimd.dma_start(out=wt[:],
                                    in_=w_d[ot * 128:(ot + 1) * 128, :, :])
                nc.vector.tensor_scalar(wt[:], wt[:], binv_col[:], RC,
                                        op0=OP.mult, op1=OP.add)
                nc.scalar.activation(wqt_all[:, ot], wt[:], AF.Sign,
                                     bias=rcn_col[:], scale=1.0)

            # ============ Pass X1: stats + |xhat| + transpose-in ==========
            # Software-pipelined groups of XG tiles: DMA+stats of group g
            # are issued before the rsqrt chain / xhat of group g-1 so the
            # in-order engine queues never convoy on the latency chain.
            xts = {}

            def x_dma_stats(g):
                for u in range(XG):
                    it = g * XG + u
                    xt = xin.tile([128, C], F32, tag="xt", name="xt")
                    xts[it] = xt
                    eng = nc.sync if it % 2 == 0 else nc.scalar
                    eng.dma_start(out=xt[:],
                                  in_=x_d[it * 128:(it + 1) * 128, :])
                    st6 = st_p.tile([128, NS, 6], F32)
                    for sb in range(NS):
                        nc.vector.bn_stats(st6[:, sb, :],
                                           xt[:, sb * SUB:(sb + 1) * SUB])
                    nc.vector.bn_aggr(mv_all[:, it, :], st6[:])

            def x_chain(g):
                # rsig = 1/sqrt(var+eps): +eps folds into the Sqrt bias so
                # the chain is one ACT op + three small DVE ops
                gs = slice(g * XG, (g + 1) * XG)
                s0 = grp_p.tile([128, XG], F32, tag="g2", name="g2")
                nc.scalar.activation(s0[:], mv_all[:, gs, 1], AF.Sqrt,
                                     bias=eps_col[:], scale=1.0)
                nc.vector.reciprocal(rsig_all[:, gs], s0[:])
                mr = grp_p.tile([128, XG], F32, tag="g2", name="g2")
                nc.vector.tensor_tensor(mr[:], mv_all[:, gs, 0],
                                        rsig_all[:, gs], op=OP.mult)
                nc.vector.tensor_scalar_mul(nmr_all[:, gs], mr[:], -1.0)

            def x_finish(g):
                for u in range(XG):
                    it = g * XG + u
                    xt = xts.pop(it)
                    xh = xhat_p.tile([128, C], F16, tag="xh", name="xh")
                    nc.scalar.activation(
                        xh[:], xt[:], AF.Identity,
                        bias=nmr_all[:, it:it + 1],
                        scale=rsig_all[:, it:it + 1])
                    nc.vector.tensor_tensor(amx_t[:], amx_t[:], xh[:],
                                            op=OP.max)
                    nc.vector.tensor_tensor(amn_t[:], amn_t[:], xh[:],
                                            op=OP.min)
                    # PE-transpose |xhat| into the bf16 operand buffer:
                    # kills the second x pass, the xq DRAM round-trip and
                    # the A broadcast; also keeps the PE warm pre-main
                    for jb2 in range(0, NCC, 4):
                        pt = ptile16()
                        for b in range(4):
                            jb = jb2 + b
                            nc.tensor.transpose(
                                pt[:, b * 128:(b + 1) * 128],
                                xh[:, jb * 128:(jb + 1) * 128], identh[:])
                        dst = xqt_all[:, jb2:jb2 + 4,
                                      XPAD + it * 128:XPAD + (it + 1) * 128]
                        if it % 2 == 0:
                            nc.vector.tensor_copy(dst, pt[:, 0:512])
                        else:
                            nc.scalar.activation(dst, pt[:, 0:512],
                                                 AF.Identity)

            x_dma_stats(0)
            x_chain(0)
            for g in range(1, NG):
                x_dma_stats(g)
                x_finish(g - 1)
                x_chain(g)
                for ot in W1_AT.get(g, []):
                    w_abs(ot)
                if g == BETA_G:
                    beta_block()
                for ot in W2_AT.get(g, []):
                    w_quant(ot)
            x_finish(NG - 1)
            for ot in W2_TAIL:
                w_quant(ot)

            # ============ per-channel scales (column form) ================
            Mx = small.tile([128, NCC], F32, tag="Mx", name="Mx")
            Mn = small.tile([128, NCC], F32, tag="Mn", name="Mn")
            for j in range(NCC):
                pmx = ptile16()
                nc.tensor.transpose(pmx[:, 0:128],
                                    amx_t[:, j * 128:(j + 1) * 128], identh[:])
                nc.vector.tensor_reduce(Mx[:, j:j + 1], pmx[:, 0:128],
                                        axis=AX.X, op=OP.max)
                pmn = ptile16()
                nc.tensor.transpose(pmn[:, 0:128],
                                    amn_t[:, j * 128:(j + 1) * 128],
                                    identh[:])
                nc.vector.tensor_reduce(Mn[:, j:j + 1], pmn[:, 0:128],
                                        axis=AX.X, op=OP.min)
            # batched endpoint math on [128, NCC]
            t1 = small.tile([128, NCC], F32, tag="t1", name="t1")
            t2 = small.tile([128, NCC], F32, tag="t2", name="t2")
            nc.vector.tensor_tensor(t1[:], g_mat[:], Mx[:], op=OP.mult)
            nc.vector.tensor_tensor(t2[:], g_mat[:], Mn[:], op=OP.mult)
            if not beta_zero:
                nc.vector.tensor_tensor(t1[:], t1[:], b_mat[:], op=OP.add)
                nc.vector.tensor_tensor(t2[:], t2[:], b_mat[:], op=OP.add)
            m1 = small.tile([128, NCC], F32, tag="m1", name="m1")
            nc.vector.tensor_tensor(m1[:], t1[:], t2[:], op=OP.max)
            nc.vector.tensor_scalar_mul(t2[:], t2[:], -1.0)
            nc.vector.tensor_scalar_mul(t1[:], t1[:], -1.0)
            nc.vector.tensor_tensor(m1[:], m1[:], t2[:], op=OP.max)
            nc.vector.tensor_tensor(m1[:], m1[:], t1[:], op=OP.max)  # amax
            nc.vector.tensor_scalar_max(m1[:], m1[:], EPS_CLAMP)     # gamma_q
            ginv = small.tile([128, NCC], F32, tag="ginv", name="ginv")
            nc.vector.reciprocal(ginv[:], m1[:])
            sc_m = small.tile([128, NCC], F32, tag="scm", name="scm")
            nc.vector.tensor_scalar_mul(sc_m[:], ginv[:], QP)
            scinv = small.tile([128, NCC], F32, tag="sci", name="sci")
            nc.vector.reciprocal(scinv[:], sc_m[:])
            # A (and B) are per-partition columns in the transposed domain
            A_m = small.tile([128, NCC], F32, tag="Am", name="Am")
            nc.vector.tensor_tensor(A_m[:], g_mat[:], sc_m[:], op=OP.mult)
            r_m = small.tile([128, NCC], F32, tag="rm", name="rm")
            nc.vector.tensor_scalar_mul(r_m[:], scinv[:], beta_col[:])
            BR_m = small.tile([128, NCC], F32, tag="BRm", name="BRm")
            if not beta_zero:
                nc.vector.tensor_tensor(BR_m[:], b_mat[:], sc_m[:],
                                        op=OP.mult)
                nc.vector.tensor_scalar_add(BR_m[:], BR_m[:], RC)
            # broadcast r to [128, C] via per-column transpose + K=1 matmul
            for j in range(NCC):
                prow = ptile()
                nc.tensor.transpose(prow[0:1, 0:128], r_m[:, j:j + 1],
                                    identb[:])
                rw = small.tile([1, 128], F32, tag="rw", name="rw")
                nc.vector.tensor_copy(rw[:], prow[0:1, 0:128])
                pbc = ptile()
                nc.tensor.matmul(pbc[:, 0:128], ones_row[:], rw[:],
                                 start=True, stop=True)
                nc.vector.tensor_copy(r_b[:, j * 128:(j + 1) * 128],
                                      pbc[:, 0:128])

            # zero the one-column halo each side of the time axis
            for j in range(NCC):
                nc.vector.memset(xqt_all[:, j, XPAD - 1:XPAD], 0.0)
                nc.vector.memset(xqt_all[:, j, XPAD + T:XPAD + T + 1], 0.0)

            # ============ quantize chunks + conv matmuls ==================
            def quantize(q):
                # in place over xqt_all: u = xhat*A (+B) + RC on ACT;
                # xq = u - RC on DVE (bf16 out)
                for j in range(NCC):
                    sl = xqt_all[:, j, XPAD + q * TQ:XPAD + (q + 1) * TQ]
                    u = u_p.tile([128, TQ], F32, tag="u", name="u")
                    nc.scalar.activation(
                        u[:], sl, AF.Identity,
                        bias=rcp_col[:] if beta_zero else BR_m[:, j:j + 1],
                        scale=A_m[:, j:j + 1])
                    nc.vector.tensor_scalar_add(sl, u[:], -RC)

            def consume(q):
                for itq in range(NTQ):
                    it = q * NTQ + itq
                    pss = [psum_mm.tile([128, OSL], F32, tag="mm", name="pmm")
                           for _ in range(NH)]
                    for j in range(NCC):
                        for k in range(KW):
                            lhsT = xqt_all[:, j, XPAD + it * 128 + k - 1:
                                           XPAD + it * 128 + k - 1 + 128]
                            first = (j == 0 and k == 0)
                            last = (j == NCC - 1 and k == KW - 1)
                            for h in range(NH):
                                nc.tensor.matmul(
                                    pss[h][:], lhsT,
                                    wqt_all[:, j, k, h * OSL:(h + 1) * OSL],
                                    start=first, stop=last)
                    for h in range(NH):
                        yt = yout.tile([128, OSL], F32, tag="yt", name="yt")
                        nc.vector.tensor_tensor(
                            yt[:], pss[h][:], r_b[:, h * OSL:(h + 1) * OSL],
                            op=OP.mult)
                        nc.gpsimd.dma_start(
                            out=out_d[it * 128:(it + 1) * 128,
                                      h * OSL:(h + 1) * OSL],
                            in_=yt[:])

            quantize(0)
            if NQ > 1:
                quantize(1)
            for q in range(NQ):
                consume(q)
                if q + 2 < NQ:
                    quantize(q + 2)

    nc.compile()
    return nc


_NC_CACHE = {}


def _get_nc(T, C, beta_zero):
    key = (T, C, beta_zero)
    if key not in _NC_CACHE:
        _NC_CACHE[key] = build_kernel(T, C, beta_zero)
    return _NC_CACHE[key]


def run(inputs, trace=False):
    """Run the SPMD kernel; returns (output [B,T,C], BassKernelResults)."""
    x = np.ascontiguousarray(np.asarray(inputs["x"], dtype=np.float32))
    g = np.ascontiguousarray(np.asarray(inputs["ln_gamma"], dtype=np.float32))
    b = np.ascontiguousarray(np.asarray(inputs["ln_beta"], dtype=np.float32))
    W = np.asarray(inputs["W"], dtype=np.float32)
    B, T, C = x.shape
    assert B == N_CORES, f"expected batch {N_CORES}, got {B}"
    beta_zero = bool(np.all(b == 0.0))
    nc = _get_nc(T, C, beta_zero)
    # pure layout permute: supply W as [C_in, K, C_out] so the quantized
    # operand lands in matmul orientation with no on-chip transposes
    W_T = np.ascontiguousarray(W.transpose(1, 2, 0))
    in_maps = [
        {"x": np.ascontiguousarray(x[i]), "ln_gamma": g, "ln_beta": b,
         "W": W_T}
        for i in range(B)
    ]
    res = run_bass_kernel_spmd(nc, in_maps, core_ids=list(range(N_CORES)),
                               trace=trace)
    out = np.stack([res.results[i]["out"] for i in range(B)], axis=0)
    return out, res


def kernel(**inputs) -> np.ndarray:
    out, _ = run(inputs)
    return out
